# revision 6
# baseline (speedup 1.0000x reference)
"""Multi-head attention forward on 8 Trainium2 NeuronCores.

Problem: B=32, N=512, C=1024, H=16 heads, head_dim=64, fp32 I/O.
Strategy: data-parallel over batch (4 batches per core), no collectives.

HW model (microbenched): a matmul whose stationary operand (lhsT) changes
costs ~270-290 ns (weight-load-pipe bound, overlapping the rhs stream);
a matmul reusing the previous stationary streams 512 bf16 columns in
~74 ns. Weight loads to disjoint PE row groups (partition bases 0/64)
overlap. The kernel is therefore structured to maximize rhs streams per
weight load:
  - V: one xT stationary serves both 512-wide output halves (dcv pair).
  - QK: one W_qkv stationary serves two batches' xT streams (batch pair).
  - proj: one attnT stationary serves both 512-wide output halves.
  - scores: head pairs sit at partition bases 0/64 so consecutive
    matmuls' weight loads overlap in disjoint row groups.

Math notes:
  - reference adds mask[:,None,None,:] + mask[:,None,:,None] to the logits;
    the query-axis term is constant along the softmax axis so it cancels.
    The key-axis term is folded into the exp as a per-partition bias:
    e'[k,q] = exp(scale*scores^T[k,q] + mask[k]) = exp(mask)[k] * e[k,q].
  - attn@V is V-stationary: lhsT = [V_h (cols 0:64) | ones (cols 64:128)],
    rhs = e'^T[k, q] streaming all 512 q, accumulated over 4 k-tiles.
    psum rows 0:64 = unnormalized attn^T[d, q]; rows 64:128 = the softmax
    denominator replicated across 64 partitions (the ones-block broadcasts
    it for free). One DVE reciprocal + one multiply normalize directly into
    the attnT [c, q] layout the projection needs -- no PE transposes.
"""
import numpy as np
import ml_dtypes

B, N, C, H = 32, 512, 1024, 16
HD = C // H  # 64
SCALE = HD ** -0.5
NCORES = 8
BL = B // NCORES  # batches per core = 4
CT = C // 128     # 8 c-tiles
NT = N // 128     # 4 n-tiles
DC3 = 3 * C       # 3072

_cached_nc = None


def _build(repeat=1):
    import contextlib
    import concourse.mybir as mybir
    import concourse.tile as tile
    from concourse import bacc

    BF16 = mybir.dt.bfloat16
    F32 = mybir.dt.float32
    EXP = mybir.ActivationFunctionType.Exp

    nc = bacc.Bacc()
    xT_d = nc.dram_tensor("xT", [BL, C, N], BF16, kind="ExternalInput")
    wqkvT_d = nc.dram_tensor("wqkvT", [C, DC3], BF16, kind="ExternalInput")
    wprojT_d = nc.dram_tensor("wprojT", [C, C], BF16, kind="ExternalInput")
    mask_d = nc.dram_tensor("maskt", [128, BL, NT], F32, kind="ExternalInput")
    out_d = nc.dram_tensor("out", [BL, N, C], F32, kind="ExternalOutput")

    with tile.TileContext(nc) as tc:
        with (
            tc.tile_pool(name="singles", bufs=1) as singles,
            tc.tile_pool(name="xp", bufs=3) as xp,
            tc.tile_pool(name="qkp", bufs=2) as qkp,
            tc.tile_pool(name="vp", bufs=2) as vp,
            tc.tile_pool(name="ep", bufs=4) as ep,
            tc.tile_pool(name="atp", bufs=2) as atp,
            tc.tile_pool(name="rp", bufs=2) as rp,
            tc.tile_pool(name="op", bufs=3) as op,
            tc.tile_pool(name="ps_big", bufs=4, space="PSUM") as ps_big,
            tc.tile_pool(name="ps_av", bufs=2, space="PSUM") as ps_av,
        ):
            # --- one-time loads; V weight slice (cols 2C:3C) first because
            # V is computed first each batch. ---
            mask_sb = singles.tile([128, BL, NT], F32)
            nc.sync.dma_start(out=mask_sb[:], in_=mask_d[:])
            wqkvT_sb = singles.tile([128, CT, DC3], BF16)
            wqkvT_src = wqkvT_d.rearrange("(ct p) d -> p ct d", p=128)
            for j in list(range(16, 24)) + list(range(16)):
                nc.sync.dma_start(
                    out=wqkvT_sb[:, :, j * 128:(j + 1) * 128],
                    in_=wqkvT_src[:, :, j * 128:(j + 1) * 128])
            wprojT_sb = singles.tile([128, CT, C], BF16)
            nc.sync.dma_start(out=wprojT_sb[:],
                              in_=wprojT_d.rearrange("(ct p) d -> p ct d", p=128))
            # V-stationary operand pool: cols 0:64 = V_h per (kt, h),
            # rewritten each batch; cols 64:128 = constant ones. Both
            # rotating buffers get the ones block once here, so any
            # in-loop rotation still reads correct ones.
            vaug_a = vp.tile([128, NT, H, 128], BF16, tag="vaug", name="vaug_a")
            nc.vector.memset(vaug_a[:, :, :, HD:128], 1.0)
            vaug_b = vp.tile([128, NT, H, 128], BF16, tag="vaug", name="vaug_b")
            nc.vector.memset(vaug_b[:, :, :, HD:128], 1.0)

            rep_ctx = tc.For_i(0, repeat, 1) if repeat > 1 else contextlib.nullcontext()
            with rep_ctx:
              for bp in range(BL // 2):
                b0, b1 = 2 * bp, 2 * bp + 1
                xT_bufs = {}
                for bb in (b0, b1):
                    xT_bufs[bb] = xp.tile([128, CT, N], BF16, tag="xT",
                                          name=f"xTl{bb}")
                    nc.sync.dma_start(
                        out=xT_bufs[bb][:],
                        in_=xT_d[bb].rearrange("(ct p) n -> p ct n", p=128))

                # --- QK for BOTH batches: one W stationary, two streams ---
                qkT = {b0: qkp.tile([128, 16, N], BF16, tag="qkT", name="qkT0"),
                       b1: qkp.tile([128, 16, N], BF16, tag="qkT", name="qkT1")}

                def emit_qk_pair(dct):
                    ps0 = ps_big.tile([128, 512], F32, tag="big")
                    ps1 = ps_big.tile([128, 512], F32, tag="big")
                    for ct in range(CT):
                        w_ap = wqkvT_sb[:, ct, dct * 128:(dct + 1) * 128]
                        nc.tensor.matmul(ps0[:], w_ap, xT_bufs[b0][:, ct, :],
                                         start=(ct == 0), stop=(ct == CT - 1))
                        nc.tensor.matmul(ps1[:], w_ap, xT_bufs[b1][:, ct, :],
                                         start=(ct == 0), stop=(ct == CT - 1))
                    nc.vector.tensor_copy(out=qkT[b0][:, dct, :], in_=ps0[:])
                    nc.vector.tensor_copy(out=qkT[b1][:, dct, :], in_=ps1[:])

                def emit_v(b, vaug_sb):
                    # V natural [n, dc]: one xT stationary, two dcv streams
                    for nt in range(NT):
                        ps0 = ps_big.tile([128, 512], F32, tag="big")
                        ps1 = ps_big.tile([128, 512], F32, tag="big")
                        for ct in range(CT):
                            x_ap = xT_bufs[b][:, ct, nt * 128:(nt + 1) * 128]
                            nc.tensor.matmul(
                                ps0[:], x_ap,
                                wqkvT_sb[:, ct, 2 * C:2 * C + 512],
                                start=(ct == 0), stop=(ct == CT - 1))
                            nc.tensor.matmul(
                                ps1[:], x_ap,
                                wqkvT_sb[:, ct, 2 * C + 512:3 * C],
                                start=(ct == 0), stop=(ct == CT - 1))
                        nc.vector.tensor_copy(
                            out=vaug_sb[:, nt, 0:8, 0:HD],
                            in_=ps0.rearrange("p (h d) -> p h d", d=HD))
                        nc.vector.tensor_copy(
                            out=vaug_sb[:, nt, 8:16, 0:HD],
                            in_=ps1.rearrange("p (h d) -> p h d", d=HD))

                def emit_scores(b, h, eT_tiles):
                    dct_q = h // 2
                    dct_k = 8 + h // 2
                    po = (h % 2) * HD
                    eT_sb = ep.tile([128, NT, N], BF16, tag="eT")
                    for kt in range(NT):
                        ps = ps_big.tile([128, 512], F32, tag="big")
                        nc.tensor.matmul(
                            ps[:],
                            qkT[b][po:po + HD, dct_k, kt * 128:(kt + 1) * 128],
                            qkT[b][po:po + HD, dct_q, :],
                            start=True, stop=True)
                        nc.scalar.activation(eT_sb[:, kt, :], ps[:], EXP,
                                             bias=mask_sb[:, b, kt:kt + 1],
                                             scale=SCALE)
                    eT_tiles[h] = eT_sb

                def emit_attnv(h, eT_tiles, attnT_sb, vaug_sb):
                    eT_sb = eT_tiles.pop(h)
                    ps = ps_av.tile([128, N], F32, tag="av")
                    for kt in range(NT):
                        nc.tensor.matmul(
                            ps[:],
                            vaug_sb[:, kt, h, :],
                            eT_sb[:, kt, :],
                            start=(kt == 0), stop=(kt == NT - 1))
                    rt = rp.tile([128, N], F32, tag="recip")
                    nc.vector.reciprocal(rt[HD:128, :], ps[HD:128, :])
                    nc.vector.tensor_mul(
                        attnT_sb[(h % 2) * HD:(h % 2) * HD + HD, h // 2, :],
                        ps[0:HD, :], rt[HD:128, :])

                def emit_proj(b, attnT_sb):
                    # one attnT stationary, two 512-wide output streams
                    for qt in range(NT):
                        out_sb = op.tile([128, C], F32, tag="out")
                        ps0 = ps_big.tile([128, 512], F32, tag="big")
                        ps1 = ps_big.tile([128, 512], F32, tag="big")
                        for ct in range(CT):
                            a_ap = attnT_sb[:, ct, qt * 128:(qt + 1) * 128]
                            nc.tensor.matmul(ps0[:], a_ap,
                                             wprojT_sb[:, ct, 0:512],
                                             start=(ct == 0), stop=(ct == CT - 1))
                            nc.tensor.matmul(ps1[:], a_ap,
                                             wprojT_sb[:, ct, 512:C],
                                             start=(ct == 0), stop=(ct == CT - 1))
                        nc.vector.tensor_copy(out=out_sb[:, 0:512], in_=ps0[:])
                        nc.vector.tensor_copy(out=out_sb[:, 512:C], in_=ps1[:])
                        nc.sync.dma_start(out=out_d[b, qt * 128:(qt + 1) * 128, :],
                                          in_=out_sb[:])

                def emit_batch_inner(b, vaug_sb):
                    # scores -> exp -> attn@V -> normalize -> proj for one batch
                    attnT_sb = atp.tile([128, CT, N], BF16, tag="attnT")
                    eT_tiles = {}
                    for j in range(8):
                        emit_scores(b, 2 * j, eT_tiles)
                        emit_scores(b, 2 * j + 1, eT_tiles)
                        if j > 0:
                            emit_attnv(2 * j - 2, eT_tiles, attnT_sb, vaug_sb)
                            emit_attnv(2 * j - 1, eT_tiles, attnT_sb, vaug_sb)
                    emit_attnv(H - 2, eT_tiles, attnT_sb, vaug_sb)
                    emit_attnv(H - 1, eT_tiles, attnT_sb, vaug_sb)
                    emit_proj(b, attnT_sb)

                vaug0 = vp.tile([128, NT, H, 128], BF16, tag="vaug",
                                name=f"vaug{b0}")
                emit_v(b0, vaug0)
                for dct in range(16):
                    emit_qk_pair(dct)
                emit_batch_inner(b0, vaug0)
                vaug1 = vp.tile([128, NT, H, 128], BF16, tag="vaug",
                                name=f"vaug{b1}")
                emit_v(b1, vaug1)
                emit_batch_inner(b1, vaug1)
    nc.finalize()
    return nc


def _prep_inputs(x, mask, W_qkv, W_proj):
    bf16 = ml_dtypes.bfloat16
    xT = np.ascontiguousarray(x.transpose(0, 2, 1)).astype(bf16)      # [B, C, N]
    wqkvT = np.ascontiguousarray(W_qkv.T).astype(bf16)                # [C, 3C]
    wprojT = np.ascontiguousarray(W_proj.T).astype(bf16)              # [C, C]
    # raw mask pre-striped for SBUF layout [p, b_local, nt]: mask[b, nt*128+p]
    mask_striped = np.ascontiguousarray(
        mask.astype(np.float32).reshape(B, NT, 128).transpose(2, 0, 1))
    return [
        {
            "xT": xT[c * BL:(c + 1) * BL],
            "wqkvT": wqkvT,
            "wprojT": wprojT,
            "maskt": mask_striped[:, c * BL:(c + 1) * BL, :],
        }
        for c in range(NCORES)
    ]


def build_null():
    """Null kernel with the same external I/O — timing baseline."""
    import concourse.mybir as mybir
    import concourse.tile as tile
    from concourse import bacc

    BF16 = mybir.dt.bfloat16
    F32 = mybir.dt.float32
    nc = bacc.Bacc()
    nc.dram_tensor("xT", [BL, C, N], BF16, kind="ExternalInput")
    nc.dram_tensor("wqkvT", [C, DC3], BF16, kind="ExternalInput")
    nc.dram_tensor("wprojT", [C, C], BF16, kind="ExternalInput")
    mask_d = nc.dram_tensor("maskt", [128, BL, NT], F32, kind="ExternalInput")
    out_d = nc.dram_tensor("out", [BL, N, C], F32, kind="ExternalOutput")
    with tile.TileContext(nc) as tc:
        with tc.tile_pool(name="sb", bufs=1) as sb:
            t = sb.tile([128, 4], F32)
            nc.sync.dma_start(out=t[:], in_=mask_d[:, 0, :])
            nc.sync.dma_start(out=out_d[0, 0:128, 0:4], in_=t[:])
    nc.finalize()
    return nc


def get_nc():
    global _cached_nc
    if _cached_nc is None:
        _cached_nc = _build()
    return _cached_nc


def kernel(x, mask, W_qkv, W_proj, b_proj):
    from concourse.bass_utils import run_bass_kernel_spmd

    nc = get_nc()
    in_maps = _prep_inputs(np.asarray(x, dtype=np.float32),
                           np.asarray(mask, dtype=np.float32),
                           np.asarray(W_qkv, dtype=np.float32),
                           np.asarray(W_proj, dtype=np.float32))
    res = run_bass_kernel_spmd(nc, in_maps, core_ids=list(range(NCORES)))
    out = np.concatenate([res.results[c]["out"] for c in range(NCORES)], axis=0)
    out = out + np.asarray(b_proj, dtype=np.float32)[None, None, :]
    return np.ascontiguousarray(out.astype(np.float32))


# revision 8
# speedup vs baseline: 1.1609x; 1.1609x over previous
"""Multi-head attention forward on 8 Trainium2 NeuronCores.

Problem: B=32, N=512, C=1024, H=16 heads, head_dim=64, fp32 I/O.
Strategy: data-parallel over batch (4 batches per core), no collectives.

Math notes:
  - reference adds mask[:,None,None,:] + mask[:,None,:,None] to the logits;
    the query-axis term is constant along the softmax axis so it cancels.
    The key-axis term is folded into the exp as a per-partition bias:
    e'[k,q] = exp(scale*scores^T[k,q] + mask[k]) = exp(mask)[k] * e[k,q].
  - attn@V is V-stationary: lhsT = [V_h (cols 0:64) | ones (cols 64:128)],
    rhs = e'^T[k, q] streaming all 512 q, accumulated over 4 k-tiles.
    psum rows 0:64 = unnormalized attn^T[d, q]; rows 64:128 = the softmax
    denominator replicated across 64 partitions (the ones-block broadcasts
    it for free). One DVE reciprocal + one multiply normalize directly into
    the attnT [c, q] layout the projection needs -- no PE transposes, and
    every matmul streams >=512 columns so LDWEIGHTS hides under the stream.

Layouts (per core, per batch):
  xT [c,n] (host pre-transposed) -> Q^T,K^T [dc,n] via W-stationary matmuls,
  V [n,dc] via x-stationary matmuls (unscaled).
  scores^T [k,q] per head (contract d=64). Head pairs sit at partitions
  0:64 / 64:128, so their matmuls auto-derive tile_position (0,0)/(64,0)
  and overlap in disjoint PE row groups on hardware.
  exp+mask-bias on ScalarE -> e'^T bf16; attn^T as above;
  proj: lhsT=attn^T c-tiles, rhs=W_proj^T -> out [q, c].
"""
import numpy as np
import ml_dtypes

B, N, C, H = 32, 512, 1024, 16
HD = C // H  # 64
SCALE = HD ** -0.5
NCORES = 8
BL = B // NCORES  # batches per core = 4
CT = C // 128     # 8 c-tiles
NT = N // 128     # 4 n-tiles
DC3 = 3 * C       # 3072

_cached_nc = None


def _build(repeat=1):
    import contextlib
    import concourse.mybir as mybir
    import concourse.tile as tile
    from concourse import bacc

    BF16 = mybir.dt.bfloat16
    F32 = mybir.dt.float32
    EXP = mybir.ActivationFunctionType.Exp

    nc = bacc.Bacc()
    xT_d = nc.dram_tensor("xT", [BL, C, N], BF16, kind="ExternalInput")
    wqkvT_d = nc.dram_tensor("wqkvT", [C, DC3], BF16, kind="ExternalInput")
    wprojT_d = nc.dram_tensor("wprojT", [C, C], BF16, kind="ExternalInput")
    mask_d = nc.dram_tensor("maskt", [128, BL, NT], F32, kind="ExternalInput")
    out_d = nc.dram_tensor("out", [BL, N, C], F32, kind="ExternalOutput")

    with tile.TileContext(nc) as tc:
        with (
            tc.tile_pool(name="singles", bufs=1) as singles,
            tc.tile_pool(name="xp", bufs=2) as xp,
            tc.tile_pool(name="qkp", bufs=2) as qkp,
            tc.tile_pool(name="ep", bufs=4) as ep,
            tc.tile_pool(name="atp", bufs=2) as atp,
            tc.tile_pool(name="rp", bufs=3) as rp,
            tc.tile_pool(name="op", bufs=3) as op,
            tc.tile_pool(name="ps_big", bufs=4, space="PSUM") as ps_big,
            tc.tile_pool(name="ps_av", bufs=3, space="PSUM") as ps_av,
        ):
            # --- one-time loads, ordered so the first QKV group unblocks
            # after ~1.25 MB: x^T(b=0) + the V weight slice. Weights are
            # chunked along the OUTPUT axis; V columns (2C:3C) come first
            # because V is computed first each batch. ---
            mask_sb = singles.tile([128, BL, NT], F32)
            nc.sync.dma_start(out=mask_sb[:], in_=mask_d[:])
            xT_first = xp.tile([128, CT, N], BF16, tag="xT")
            nc.sync.dma_start(out=xT_first[:],
                              in_=xT_d[0].rearrange("(ct p) n -> p ct n", p=128))
            wqkvT_sb = singles.tile([128, CT, DC3], BF16)
            wqkvT_src = wqkvT_d.rearrange("(ct p) d -> p ct d", p=128)
            for j in list(range(16, 24)) + list(range(16)):
                nc.sync.dma_start(
                    out=wqkvT_sb[:, :, j * 128:(j + 1) * 128],
                    in_=wqkvT_src[:, :, j * 128:(j + 1) * 128])
            wprojT_sb = singles.tile([128, CT, C], BF16)
            nc.sync.dma_start(out=wprojT_sb[:],
                              in_=wprojT_d.rearrange("(ct p) d -> p ct d", p=128))
            # V-stationary operand: cols 0:64 = V_h per (kt, h), rewritten
            # each batch; cols 64:128 = constant ones (set once).
            vaug_sb = singles.tile([128, NT, H, 128], BF16)
            nc.vector.memset(vaug_sb[:, :, :, HD:128], 1.0)

            rep_ctx = tc.For_i(0, repeat, 1) if repeat > 1 else contextlib.nullcontext()
            with rep_ctx:
              for b in range(BL):
                # --- load x^T for this batch (b=0 preloaded above) ---
                if b == 0:
                    xT_sb = xT_first
                else:
                    xT_sb = xp.tile([128, CT, N], BF16, tag="xT")
                    nc.sync.dma_start(
                        out=xT_sb[:],
                        in_=xT_d[b].rearrange("(ct p) n -> p ct n", p=128))

                # --- V natural [n, dc] FIRST so attn@V can fuse into the
                # QK/scores loop below ---
                for nt in range(NT):
                    for dcv in range(2):
                        ps = ps_big.tile([128, 512], F32, tag="big")
                        for ct in range(CT):
                            nc.tensor.matmul(
                                ps[:],
                                xT_sb[:, ct, nt * 128:(nt + 1) * 128],
                                wqkvT_sb[:, ct, 2 * C + dcv * 512:2 * C + (dcv + 1) * 512],
                                start=(ct == 0), stop=(ct == CT - 1))
                        nc.vector.tensor_copy(
                            out=vaug_sb[:, nt, 8 * dcv:8 * (dcv + 1), 0:HD],
                            in_=ps.rearrange("p (h d) -> p h d", d=HD))

                qkT_sb = qkp.tile([128, 16, N], BF16, tag="qkT")

                def emit_qk(dct):
                    ps = ps_big.tile([128, 512], F32, tag="big")
                    for ct in range(CT):
                        nc.tensor.matmul(
                            ps[:],
                            wqkvT_sb[:, ct, dct * 128:(dct + 1) * 128],
                            xT_sb[:, ct, :],
                            start=(ct == 0), stop=(ct == CT - 1))
                    nc.vector.tensor_copy(out=qkT_sb[:, dct, :], in_=ps[:])

                attnT_sb = atp.tile([128, CT, N], BF16, tag="attnT")
                eT_tiles = {}

                def emit_scores(h):
                    dct_q = h // 2
                    dct_k = 8 + h // 2
                    po = (h % 2) * HD
                    eT_sb = ep.tile([128, NT, N], BF16, tag="eT")
                    for kt in range(NT):
                        ps = ps_big.tile([128, 512], F32, tag="big")
                        nc.tensor.matmul(
                            ps[:],
                            qkT_sb[po:po + HD, dct_k, kt * 128:(kt + 1) * 128],
                            qkT_sb[po:po + HD, dct_q, :],
                            start=True, stop=True)
                        nc.scalar.activation(eT_sb[:, kt, :], ps[:], EXP,
                                             bias=mask_sb[:, b, kt:kt + 1],
                                             scale=SCALE)
                    eT_tiles[h] = eT_sb

                def emit_attnv(h):
                    eT_sb = eT_tiles.pop(h)
                    ps = ps_av.tile([128, N], F32, tag="av")
                    for kt in range(NT):
                        nc.tensor.matmul(
                            ps[:],
                            vaug_sb[:, kt, h, :],
                            eT_sb[:, kt, :],
                            start=(kt == 0), stop=(kt == NT - 1))
                    rt = rp.tile([128, N], F32, tag="recip")
                    nc.vector.reciprocal(rt[HD:128, :], ps[HD:128, :])
                    nc.vector.tensor_mul(
                        attnT_sb[(h % 2) * HD:(h % 2) * HD + HD, h // 2, :],
                        ps[0:HD, :], rt[HD:128, :])

                # Fused schedule: QK pair j -> attn@V for pair j-1 (fills the
                # PE while DVE evacuates qkT) -> scores/exp for heads 2j,2j+1.
                for j in range(8):
                    emit_qk(j)
                    emit_qk(8 + j)
                    if j > 0:
                        emit_attnv(2 * j - 2)
                        emit_attnv(2 * j - 1)
                    emit_scores(2 * j)
                    emit_scores(2 * j + 1)
                emit_attnv(H - 2)
                emit_attnv(H - 1)

                # --- projection ---
                for qt in range(NT):
                    out_sb = op.tile([128, C], F32, tag="out")
                    for cot in range(2):
                        ps = ps_big.tile([128, 512], F32, tag="big")
                        for ct in range(CT):
                            nc.tensor.matmul(
                                ps[:],
                                attnT_sb[:, ct, qt * 128:(qt + 1) * 128],
                                wprojT_sb[:, ct, cot * 512:(cot + 1) * 512],
                                start=(ct == 0), stop=(ct == CT - 1))
                        nc.vector.tensor_copy(out=out_sb[:, cot * 512:(cot + 1) * 512], in_=ps[:])
                        nc.sync.dma_start(
                            out=out_d[b, qt * 128:(qt + 1) * 128,
                                      cot * 512:(cot + 1) * 512],
                            in_=out_sb[:, cot * 512:(cot + 1) * 512])
    nc.finalize()
    return nc


def _prep_inputs(x, mask, W_qkv, W_proj):
    bf16 = ml_dtypes.bfloat16
    xT = np.ascontiguousarray(x.transpose(0, 2, 1)).astype(bf16)      # [B, C, N]
    wqkvT = np.ascontiguousarray(W_qkv.T).astype(bf16)                # [C, 3C]
    wprojT = np.ascontiguousarray(W_proj.T).astype(bf16)              # [C, C]
    # raw mask pre-striped for SBUF layout [p, b_local, nt]: mask[b, nt*128+p]
    mask_striped = np.ascontiguousarray(
        mask.astype(np.float32).reshape(B, NT, 128).transpose(2, 0, 1))
    return [
        {
            "xT": xT[c * BL:(c + 1) * BL],
            "wqkvT": wqkvT,
            "wprojT": wprojT,
            "maskt": mask_striped[:, c * BL:(c + 1) * BL, :],
        }
        for c in range(NCORES)
    ]


def build_null():
    """Null kernel with the same external I/O — timing baseline."""
    import concourse.mybir as mybir
    import concourse.tile as tile
    from concourse import bacc

    BF16 = mybir.dt.bfloat16
    F32 = mybir.dt.float32
    nc = bacc.Bacc()
    nc.dram_tensor("xT", [BL, C, N], BF16, kind="ExternalInput")
    nc.dram_tensor("wqkvT", [C, DC3], BF16, kind="ExternalInput")
    nc.dram_tensor("wprojT", [C, C], BF16, kind="ExternalInput")
    mask_d = nc.dram_tensor("maskt", [128, BL, NT], F32, kind="ExternalInput")
    out_d = nc.dram_tensor("out", [BL, N, C], F32, kind="ExternalOutput")
    with tile.TileContext(nc) as tc:
        with tc.tile_pool(name="sb", bufs=1) as sb:
            t = sb.tile([128, 4], F32)
            nc.sync.dma_start(out=t[:], in_=mask_d[:, 0, :])
            nc.sync.dma_start(out=out_d[0, 0:128, 0:4], in_=t[:])
    nc.finalize()
    return nc


def get_nc():
    global _cached_nc
    if _cached_nc is None:
        _cached_nc = _build()
    return _cached_nc


def kernel(x, mask, W_qkv, W_proj, b_proj):
    from concourse.bass_utils import run_bass_kernel_spmd

    nc = get_nc()
    in_maps = _prep_inputs(np.asarray(x, dtype=np.float32),
                           np.asarray(mask, dtype=np.float32),
                           np.asarray(W_qkv, dtype=np.float32),
                           np.asarray(W_proj, dtype=np.float32))
    res = run_bass_kernel_spmd(nc, in_maps, core_ids=list(range(NCORES)))
    out = np.concatenate([res.results[c]["out"] for c in range(NCORES)], axis=0)
    out = out + np.asarray(b_proj, dtype=np.float32)[None, None, :]
    return np.ascontiguousarray(out.astype(np.float32))
